# revision 3
# baseline (speedup 1.0000x reference)
"""Trainium2 Bass kernel for a discriminative (instance-segmentation) loss.

Math (per batch b, with E=64-dim embeddings, K=32 clusters, N=4096 points):
  centroids C[k] = sum_n masks[n,k]*emb[n] / msum[k]
  L_v = mean_b sum_n relu(||emb_n - C_own(n)|| - 0.5)^2 / N
  L_d = mean_b sum_{k!=j} relu(3 - ||C_k - C_j||)^2 / (K*(K-1))
  L_r = mean_b mean_k ||C_k||
  loss = L_v + L_d + 0.001 * L_r

Sharding: data-parallel over the batch dim (B=8 -> 8 NeuronCores, one batch
each).  Each core computes its per-batch scalar; the host averages the 8
scalars.

Per-core layout: n = 32*p + c  (p = SBUF partition 0..127, c = chunk 0..31),
so each partition's slice of `emb`/`masks` is one contiguous DRAM block
(line-rate DMA descriptors).  Chunks are processed in 8 groups of 4.

v2 structure (see git history for the phase-separated v1):
  0. DMA triggers first (msk on ACT queue, emb on SP queue) so transfers
     start at the earliest possible point; a single sqrt_and_others ACT
     table load warms concurrently; ~N_WARM dummy matmuls keep the PE busy
     through the DMA window so the HAM clock gate is at 2.4GHz (not the
     1.2GHz cold rate) when real matmuls start.
  1. PE: 8 group transposes masksT (+msum partial col via the ones column)
  2. PE: Cu accumulation as 16 chunk-pair matmuls into ONE [64,128] psum
     (TL/BR diag blocks hold even/odd-chunk partial sums); 2 tiny
     extraction matmuls fold TL+BR -> Cu [32,64]
  3. msum -> 1/msum; C = Cu*recip; cn2 = ||C_k||^2 (ACT square+accum)
  4. PE: C_own per group via block-diag trick, minus emb via -I matmul
  5. per group: ACT square, DVE reduce -> dist2 [p,4]; ACT sqrt; DVE hinge;
     ACT square+accum into vacc[:,g]  (pipelined behind phase 4's PE work)
  6. tiny [32,32] centroid-pairwise hinge + mean-norm tail (concurrent)
  7. final reduce + one [128,1]x[128,1] matmul -> scalar -> DMA out

Inputs are fed in bf16 (masks are exactly representable; emb rounding is
~1e-5 of the loss) which halves DMA bytes and runs the PE at 1 cycle/col.
All accumulation stays fp32 (PSUM + DVE/ACT).

NOTE: InstTensorTensorReduce crashes the device on this path -- use
separate mul/square + reduce instead.
"""

from contextlib import ExitStack

import numpy as np
import ml_dtypes

import concourse.bass as bass
import concourse.bacc as bacc
import concourse.tile as tile
from concourse import mybir
from concourse import bass_utils

F32 = mybir.dt.float32
BF16 = mybir.dt.bfloat16
AX = mybir.AxisListType
OP = mybir.AluOpType
AF = mybir.ActivationFunctionType

B, N, E, K = 8, 4096, 64, 32
P = 128            # SBUF partitions; n = 32*p + c
CHUNKS = N // P    # 32
GROUPS = 8         # 4 chunks per group
CPG = CHUNKS // GROUPS  # 4
NPAIR = CHUNKS // 2     # 16
DELTA_V = 0.5
DELTA_D = 1.5
ALPHA, BETA, GAMMA = 1.0, 1.0, 0.001
N_WARM = 26        # dummy matmuls that hold the PE busy through the DMA wait

# const pack columns (bf16): [I_128 | ones | stackedI_32 | (1 - I_32) | stackedI^T]
CP_ID = 0          # id129: cols 0..128 inclusive of ones col
CP_STKI = P + 1    # 129..160
CP_EYEC = P + 1 + K  # 161..192
CP_STKIT = P + 1 + 2 * K  # 193..320 (rows 0:32 valid)
CP_NEGI = P + 1 + 2 * K + P  # 321..448: -I_128 (PE-side subtraction)
CP_W = P + 1 + 2 * K + 2 * P


def _body(nc, tc, ctx, t, stage):
    """Emit the kernel body. `stage` < 99 stops early and DMAs an
    intermediate to the debug output (bisection aid)."""
    consts = ctx.enter_context(tc.tile_pool(name="consts", bufs=1))
    big = ctx.enter_context(tc.tile_pool(name="big", bufs=1))
    work = ctx.enter_context(tc.tile_pool(name="work", bufs=3))
    small = ctx.enter_context(tc.tile_pool(name="small", bufs=1))
    p_cu = ctx.enter_context(tc.tile_pool(name="p_cu", bufs=1, space="PSUM"))
    p_mt = ctx.enter_context(tc.tile_pool(name="p_mt", bufs=2, space="PSUM"))
    p_2 = ctx.enter_context(tc.tile_pool(name="p_2", bufs=3, space="PSUM"))
    p_sm = ctx.enter_context(tc.tile_pool(name="p_sm", bufs=2, space="PSUM"))

    def dbg(ap):
        rows, cols = ap.shape[0], int(np.prod(ap.shape[1:]))
        flat = ap if len(ap.shape) == 2 else ap.rearrange("p ... -> p (...)")
        tmp = small.tile([rows, cols], F32, tag="dbgtmp")
        nc.scalar.copy(tmp, flat)
        nc.sync.dma_start(out=t["dbg"][0:rows, 0:cols], in_=tmp)

    # ---- input loads first: msk on the ACT ring, emb on the SP ring ----
    emb_sb = big.tile([P, CHUNKS * E], BF16)       # [p, 64*c + e]
    msk_sb = big.tile([P, CHUNKS, K], BF16)        # [p, c, k]
    nc.scalar.dma_start(
        out=msk_sb, in_=t["msk"][:, :].rearrange("(p c) k -> p c k", p=P)
    )
    nc.sync.dma_start(
        out=emb_sb, in_=t["emb"][:, :].rearrange("(p c) e -> p (c e)", p=P)
    )
    cpack = consts.tile([P, CP_W], BF16)
    nc.scalar.dma_start(out=cpack, in_=t["cpack"][:, :])
    id129 = cpack[:, CP_ID:CP_ID + P + 1]
    stki = cpack[:, CP_STKI:CP_STKI + K]
    eyec = cpack[0:K, CP_EYEC:CP_EYEC + K]
    stkit = cpack[0:K, CP_STKIT:CP_STKIT + P]
    negi = cpack[:, CP_NEGI:CP_NEGI + P]

    # ---- constants / memsets ----
    ones1 = consts.tile([P, 1], F32)
    nc.vector.memset(ones1, 1.0)
    bias_m = consts.tile([K, 1], F32)     # 2*DELTA_D margin bias
    nc.vector.memset(bias_m, 2.0 * DELTA_D)
    c4bd = big.tile([P, CPG * E], BF16)   # blockdiag(C x4), filled later
    nc.gpsimd.memset(c4bd, 0.0)
    init_acc = small.tile([P, 1], F32)    # L_d/L_r row accumulator
    nc.gpsimd.memset(init_acc, 0.0)
    warm_sb = consts.tile([P, P], BF16)   # zeros for the PE warm-up matmuls
    nc.gpsimd.memset(warm_sb, 0.0)

    # warm the ACT table (Square/Sqrt/Relu/Copy all live in sqrt_and_others)
    warm = small.tile([1, 1], F32)
    nc.scalar.activation(warm, ones1[0:1, :], AF.Sqrt)

    # ---- PE warm-up: hold the array busy through the DMA window so the
    # HAM clock gate releases (1.2 -> 2.4 GHz) before the real matmuls ----
    pw = p_2.tile([P, P], F32, tag="pg")
    for _ in range(N_WARM):
        nc.tensor.matmul(pw, lhsT=warm_sb, rhs=warm_sb, start=True, stop=True)

    if stage <= 1:
        return dbg(msk_sb[:, 0:4, :])

    # ---- phase 1: masks transposes (+msum partials) ----
    mskT = big.tile([P, GROUPS, P + 1], BF16)  # [32j+k, g, p | msum partial]
    for g in range(GROUPS):
        mview = msk_sb[:, g * CPG:(g + 1) * CPG, :].rearrange("p a b -> p (a b)")
        pt = p_mt.tile([P, P + 1], F32)
        nc.tensor.matmul(pt, lhsT=mview, rhs=id129, start=True, stop=True)
        nc.vector.tensor_copy(out=mskT[:, g, :], in_=pt)
    if stage <= 2:
        return dbg(mskT[:, 0, :])

    # ---- msum -> recip (masks-only; runs while emb still streams) ----
    msum_parts = mskT[:, :, P:P + 1].rearrange("p g o -> p (g o)")  # [P, 8]
    msum_big = small.tile([P, 1], BF16)   # counts <= ~200, exact in bf16
    with nc.allow_low_precision(reason="per-group counts are small ints"):
        nc.vector.reduce_sum(out=msum_big, in_=msum_parts, axis=AX.X)
    ms_psum = p_sm.tile([K, 1], F32, tag="sm")
    nc.tensor.matmul(ms_psum, lhsT=stki, rhs=msum_big, start=True, stop=True)
    recip = small.tile([K, 1], F32)
    nc.vector.reciprocal(recip, ms_psum)
    if stage <= 4:
        return dbg(recip)

    # ---- Cu accumulation: 16 chunk-pair matmuls into one [64,128] psum.
    # lhsT = [msk_2i | msk_2i+1] [128,64], rhs = [emb_2i | emb_2i+1]
    # [128,128]; the TL [0:32,0:64] and BR [32:64,64:128] blocks hold the
    # even/odd-chunk Cu partials (cross blocks are junk). ----
    cu_psum = p_cu.tile([2 * K, P], F32)
    for i in range(NPAIR):
        nc.tensor.matmul(
            cu_psum,
            lhsT=msk_sb[:, 2 * i:2 * i + 2, :].rearrange("p a b -> p (a b)"),
            rhs=emb_sb[:, i * 2 * E:(i + 1) * 2 * E],
            start=(i == 0),
            stop=(i == NPAIR - 1),
        )
    cu_bf = small.tile([2 * K, P], BF16)
    nc.vector.tensor_copy(out=cu_bf, in_=cu_psum)
    # fold TL+BR via two row-selection matmuls ([I32;0] then [0;I32])
    c_psum = p_sm.tile([K, E], F32, tag="sm")
    nc.tensor.matmul(c_psum, lhsT=id129[0:2 * K, 0:K], rhs=cu_bf[:, 0:E],
                     start=True, stop=False)
    nc.tensor.matmul(c_psum, lhsT=id129[0:2 * K, K:2 * K], rhs=cu_bf[:, E:P],
                     start=False, stop=True)
    if stage == 44:
        return dbg(c_psum)

    # ---- C = Cu * recip; cn2 ----
    c_bf = small.tile([K, E], BF16)
    nc.vector.tensor_scalar_mul(c_bf, in0=c_psum, scalar1=recip)
    if stage == 45:
        return dbg(c_bf)
    scr_ke = small.tile([K, E], BF16)
    cn2 = small.tile([K, 1], F32)
    nc.scalar.activation(scr_ke, c_bf, AF.Square, accum_out=cn2)
    if stage <= 5:
        return dbg(c_bf)

    # ---- block-diag C: PE-replicate C 4x vertically, then 4 lane-aligned
    # copies into the diagonal blocks (partition ranges match, no DMA).
    rep_psum = p_sm.tile([P, E], F32, tag="sm")
    nc.tensor.matmul(rep_psum, lhsT=stkit, rhs=c_bf, start=True, stop=True)
    for j in range(CPG):
        dst = c4bd[j * K:(j + 1) * K, j * E:(j + 1) * E]
        src = rep_psum[j * K:(j + 1) * K, :]
        if j % 2 == 0:
            nc.vector.tensor_copy(out=dst, in_=src)
        else:
            nc.scalar.copy(out=dst, in_=src)
    if stage <= 6:
        return dbg(c4bd)

    # ---- phase 4+5: per-group diff on PE, square on ACT, reduce + hinge on
    # DVE, square-accum back on ACT; vacc[:,g] collects sum(hv^2) per group.
    vacc = small.tile([P, GROUPS], F32)
    for g in range(GROUPS):
        pg = p_2.tile([P, CPG * E], F32, tag="pg")
        nc.tensor.matmul(
            pg, lhsT=mskT[:, g, 0:P], rhs=c4bd, start=True, stop=False
        )
        nc.tensor.matmul(
            pg, lhsT=negi, rhs=emb_sb[:, g * CPG * E:(g + 1) * CPG * E],
            start=False, stop=True,
        )
        sq_g = work.tile([P, CPG * E], BF16, tag="sq")
        nc.scalar.activation(sq_g, pg, AF.Square)
        d2_g = work.tile([P, CPG], F32, tag="d2")
        nc.vector.reduce_sum(
            out=d2_g, in_=sq_g.rearrange("p (a b) -> p a b", b=E), axis=AX.X
        )
        s_g = work.tile([P, CPG], F32, tag="s")
        nc.scalar.sqrt(s_g, d2_g)
        hv_g = work.tile([P, CPG], F32, tag="hv")
        nc.vector.tensor_scalar(
            out=hv_g, in0=s_g, scalar1=DELTA_V, scalar2=0.0,
            op0=OP.subtract, op1=OP.max,
        )
        scr_g = work.tile([P, CPG], BF16, tag="scr")
        nc.scalar.activation(scr_g, hv_g, AF.Square,
                             accum_out=vacc[:, g:g + 1])
    if stage <= 8:
        return dbg(vacc)

    # ---- tiny pairwise-centroid tail (L_d, L_r); concurrent with phase 4 ----
    ct_psum = p_sm.tile([E, K], F32, tag="sm")
    nc.tensor.matmul(ct_psum, lhsT=c_bf, rhs=id129[0:K, 0:K],
                     start=True, stop=True)
    ct_sb = small.tile([E, K], BF16)
    nc.vector.tensor_copy(out=ct_sb, in_=ct_psum)
    g_psum = p_sm.tile([K, K], F32, tag="sm")
    nc.tensor.matmul(g_psum, lhsT=ct_sb, rhs=ct_sb, start=True, stop=True)
    w_sb = small.tile([K, K], BF16)
    nc.vector.tensor_scalar(
        out=w_sb, in0=g_psum, scalar1=-2.0, scalar2=cn2,
        op0=OP.mult, op1=OP.add,
    )
    wt_psum = p_sm.tile([K, K], F32, tag="sm")
    nc.tensor.matmul(wt_psum, lhsT=w_sb, rhs=id129[0:K, 0:K],
                     start=True, stop=True)
    d2_sb = small.tile([K, K], F32)
    nc.vector.tensor_scalar(
        out=d2_sb, in0=wt_psum, scalar1=cn2, scalar2=0.0,
        op0=OP.add, op1=OP.max,
    )
    d_sb = small.tile([K, K], F32)
    nc.scalar.sqrt(d_sb, d2_sb)
    h0_sb = small.tile([K, K], F32)
    nc.scalar.activation(h0_sb, d_sb, AF.Relu, bias=bias_m, scale=-1.0)
    h_sb = small.tile([K, K], F32)
    nc.vector.tensor_mul(h_sb, h0_sb, eyec)
    scr_kk = small.tile([K, K], F32)
    ld_raw = small.tile([K, 1], F32)
    nc.scalar.activation(scr_kk, h_sb, AF.Square, accum_out=ld_raw)
    cr_row = small.tile([K, 1], F32)
    nc.scalar.activation(cr_row, cn2, AF.Sqrt, scale=(GAMMA / K) ** 2)
    nc.vector.tensor_scalar(
        out=init_acc[0:K, :], in0=ld_raw, scalar1=BETA / float(K * (K - 1)),
        scalar2=cr_row, op0=OP.mult, op1=OP.add,
    )
    if stage <= 7:
        return dbg(init_acc)

    # ---- final reduction ----
    vsum = small.tile([P, 1], F32)
    nc.vector.reduce_sum(out=vsum, in_=vacc, axis=AX.X)
    tall = small.tile([P, 1], F32)
    nc.vector.tensor_scalar(
        out=tall, in0=vsum, scalar1=ALPHA / float(N),
        scalar2=init_acc, op0=OP.mult, op1=OP.add,
    )
    f_psum = p_sm.tile([1, 1], F32, tag="sm")
    nc.tensor.matmul(f_psum, lhsT=tall, rhs=ones1, start=True, stop=True)
    out_sb = small.tile([1, 1], F32)
    nc.scalar.copy(out_sb, f_psum)
    nc.sync.dma_start(out=t["out"][:, :], in_=out_sb)


def build_nc(stage=99):
    nc = bacc.Bacc("TRN2", target_bir_lowering=False, debug=False)
    t = {
        "emb": nc.dram_tensor("emb", [N, E], BF16, kind="ExternalInput"),
        "msk": nc.dram_tensor("msk", [N, K], BF16, kind="ExternalInput"),
        "cpack": nc.dram_tensor("cpack", [P, CP_W], BF16, kind="ExternalInput"),
        "out": nc.dram_tensor("out", [1, 1], F32, kind="ExternalOutput"),
    }
    if stage < 99:
        t["dbg"] = nc.dram_tensor("dbg", [P, 2048], F32, kind="ExternalOutput")

    with tile.TileContext(nc) as tc, ExitStack() as ctx:
        _body(nc, tc, ctx, t, stage)

    nc.compile()
    return nc


def host_consts():
    cpack = np.zeros((P, CP_W), dtype=ml_dtypes.bfloat16)
    cpack[:, 0:P] = np.eye(P)
    cpack[:, P] = 1.0
    cpack[:, CP_STKI:CP_STKI + K] = np.tile(np.eye(K), (CPG, 1))
    cpack[0:K, CP_EYEC:CP_EYEC + K] = 1.0 - np.eye(K)
    cpack[0:K, CP_STKIT:CP_STKIT + P] = np.tile(np.eye(K), (1, CPG))
    cpack[:, CP_NEGI:CP_NEGI + P] = -np.eye(P)
    return cpack


def make_in_maps(embedded, masks):
    emb = np.asarray(embedded).astype(ml_dtypes.bfloat16)
    msk = np.asarray(masks).astype(ml_dtypes.bfloat16)
    cpack = host_consts()
    return [
        {"emb": np.ascontiguousarray(emb[i]),
         "msk": np.ascontiguousarray(msk[i]),
         "cpack": cpack}
        for i in range(B)
    ]


_NC = None


def _get_nc():
    global _NC
    if _NC is None:
        _NC = build_nc()
    return _NC


def _install_ntff_shim():
    """Register the axon NTFF profile hook if the image's antenv lacks it."""
    import sys as _sys
    import types as _types

    try:
        from antenv.axon_hooks import get_axon_ntff_profile_hook  # noqa: F401
        return
    except ImportError:
        pass
    try:
        from trn_agent_boot.trn_boot import _ntff_profile_via_ctypes

        hook = _ntff_profile_via_ctypes("/opt/axon/libaxon_pjrt.so")
        mod = _types.ModuleType("antenv.axon_hooks")
        mod.get_axon_ntff_profile_hook = lambda: hook
        mod.set_axon_ntff_profile_hook = lambda h: None
        _sys.modules["antenv.axon_hooks"] = mod
    except Exception:
        pass


def run(embedded, masks, trace=False):
    nc = _get_nc()
    if trace:
        _install_ntff_shim()
    res = bass_utils.run_bass_kernel_spmd(
        nc, make_in_maps(embedded, masks), core_ids=list(range(B)), trace=trace
    )
    vals = np.array([r["out"][0, 0] for r in res.results], dtype=np.float64)
    return np.asarray(vals.mean(), dtype=np.float32), res


def kernel(embedded, masks, size):
    out, _ = run(embedded, masks)
    return out


# revision 7
# speedup vs baseline: 1.1975x; 1.1975x over previous
"""Trainium2 Bass kernel for a discriminative (instance-segmentation) loss.

Math (per batch b, with E=64-dim embeddings, K=32 clusters, N=4096 points):
  centroids C[k] = sum_n masks[n,k]*emb[n] / msum[k]
  L_v = mean_b sum_n relu(||emb_n - C_own(n)|| - 0.5)^2 / N
  L_d = mean_b sum_{k!=j} relu(3 - ||C_k - C_j||)^2 / (K*(K-1))
  L_r = mean_b mean_k ||C_k||
  loss = L_v + L_d + 0.001 * L_r

Sharding: data-parallel over the batch dim (B=8 -> 8 NeuronCores, one batch
each).  Each core computes its per-batch scalar; the host averages the 8
scalars.

Per-core layout: n = 32*p + c  (p = SBUF partition 0..127, c = chunk 0..31),
so each partition's slice of `emb`/`masks` is one contiguous DRAM block
(line-rate DMA descriptors).  Chunks are processed in 8 groups of 4.

v2 structure (see git history for the phase-separated v1):
  0. DMA triggers first (msk on ACT queue, emb on SP queue) so transfers
     start at the earliest possible point; a single sqrt_and_others ACT
     table load warms concurrently; ~N_WARM dummy matmuls keep the PE busy
     through the DMA window so the HAM clock gate is at 2.4GHz (not the
     1.2GHz cold rate) when real matmuls start.
  1. PE: 8 group transposes masksT (+msum partial col via the ones column)
  2. PE: Cu accumulation as 16 chunk-pair matmuls into ONE [64,128] psum
     (TL/BR diag blocks hold even/odd-chunk partial sums); 2 tiny
     extraction matmuls fold TL+BR -> Cu [32,64]
  3. msum -> 1/msum; C = Cu*recip; cn2 = ||C_k||^2 (ACT square+accum)
  4. PE: C_own per group via block-diag trick, minus emb via -I matmul
  5. per group: ACT square, DVE reduce -> dist2 [p,4]; ACT sqrt; DVE hinge;
     ACT square+accum into vacc[:,g]  (pipelined behind phase 4's PE work)
  6. tiny [32,32] centroid-pairwise hinge + mean-norm tail (concurrent)
  7. final reduce + one [128,1]x[128,1] matmul -> scalar -> DMA out

Inputs are fed in bf16 (masks are exactly representable; emb rounding is
~1e-5 of the loss) which halves DMA bytes and runs the PE at 1 cycle/col.
All accumulation stays fp32 (PSUM + DVE/ACT).

NOTE: InstTensorTensorReduce crashes the device on this path -- use
separate mul/square + reduce instead.
"""

from contextlib import ExitStack

import numpy as np
import ml_dtypes

import concourse.bass as bass
import concourse.bacc as bacc
import concourse.tile as tile
from concourse import mybir
from concourse import bass_utils

F32 = mybir.dt.float32
BF16 = mybir.dt.bfloat16
AX = mybir.AxisListType
OP = mybir.AluOpType
AF = mybir.ActivationFunctionType

B, N, E, K = 8, 4096, 64, 32
P = 128            # SBUF partitions; n = 32*p + c
CHUNKS = N // P    # 32
GROUPS = 8         # 4 chunks per group
CPG = CHUNKS // GROUPS  # 4
NPAIR = CHUNKS // 2     # 16
DELTA_V = 0.5
DELTA_D = 1.5
ALPHA, BETA, GAMMA = 1.0, 1.0, 0.001
N_WARM = 26        # dummy matmuls that hold the PE busy through the DMA wait

# const pack columns (bf16): [I_128 | ones | stackedI_32 | (1 - I_32) | stackedI^T]
CP_ID = 0          # id129: cols 0..128 inclusive of ones col
CP_STKI = P + 1    # 129..160
CP_EYEC = P + 1 + K  # 161..192
CP_STKIT = P + 1 + 2 * K  # 193..320 (rows 0:32 valid)
CP_NEGI = P + 1 + 2 * K + P  # 321..448: -I_128 (PE-side subtraction)
CP_W = P + 1 + 2 * K + 2 * P


def _body(nc, tc, ctx, t, stage):
    """Emit the kernel body. `stage` < 99 stops early and DMAs an
    intermediate to the debug output (bisection aid)."""
    consts = ctx.enter_context(tc.tile_pool(name="consts", bufs=1))
    big = ctx.enter_context(tc.tile_pool(name="big", bufs=1))
    work = ctx.enter_context(tc.tile_pool(name="work", bufs=3))
    small = ctx.enter_context(tc.tile_pool(name="small", bufs=1))
    p_cu = ctx.enter_context(tc.tile_pool(name="p_cu", bufs=1, space="PSUM"))
    p_mt = ctx.enter_context(tc.tile_pool(name="p_mt", bufs=3, space="PSUM"))
    p_2 = ctx.enter_context(tc.tile_pool(name="p_2", bufs=2, space="PSUM"))
    p_sm = ctx.enter_context(tc.tile_pool(name="p_sm", bufs=2, space="PSUM"))

    def dbg(ap):
        rows, cols = ap.shape[0], int(np.prod(ap.shape[1:]))
        flat = ap if len(ap.shape) == 2 else ap.rearrange("p ... -> p (...)")
        tmp = small.tile([rows, cols], F32, tag="dbgtmp")
        nc.scalar.copy(tmp, flat)
        nc.sync.dma_start(out=t["dbg"][0:rows, 0:cols], in_=tmp)

    # ---- input loads first: msk on the ACT ring, emb on the SP ring ----
    emb_sb = big.tile([P, CHUNKS * E], BF16)       # [p, 64*c + e]
    msk_sb = big.tile([P, CHUNKS, K], BF16)        # [p, c, k]
    cpack = consts.tile([P, CP_W], BF16)
    nc.sync.dma_start(out=cpack, in_=t["cpack"][:, :])
    nc.sync.dma_start(
        out=msk_sb, in_=t["msk"][:, :].rearrange("(p c) k -> p c k", p=P)
    )
    nc.sync.dma_start(
        out=emb_sb, in_=t["emb"][:, :].rearrange("(p c) e -> p (c e)", p=P)
    )
    id129 = cpack[:, CP_ID:CP_ID + P + 1]
    stki = cpack[:, CP_STKI:CP_STKI + K]
    eyec = cpack[0:K, CP_EYEC:CP_EYEC + K]
    stkit = cpack[0:K, CP_STKIT:CP_STKIT + P]
    negi = cpack[:, CP_NEGI:CP_NEGI + P]

    # ---- constants / memsets ----
    ones1 = consts.tile([P, 1], F32)
    nc.vector.memset(ones1, 1.0)
    bias_m = consts.tile([K, 1], F32)     # 2*DELTA_D margin bias
    nc.vector.memset(bias_m, 2.0 * DELTA_D)
    c4bd = big.tile([P, CPG * E], BF16)   # blockdiag(C x4), filled later
    nc.gpsimd.memset(c4bd, 0.0)
    init_acc = small.tile([P, 1], F32)    # L_d/L_r row accumulator
    nc.gpsimd.memset(init_acc, 0.0)
    warm_sb = consts.tile([P, P], BF16)   # zeros for the PE warm-up matmuls
    nc.gpsimd.memset(warm_sb, 0.0)

    # warm the ACT table (Square/Sqrt/Relu/Copy all live in sqrt_and_others)
    warm = small.tile([1, 1], F32)
    nc.scalar.activation(warm, ones1[0:1, :], AF.Sqrt)

    # ---- PE warm-up: hold the array busy through the DMA window so the
    # HAM clock gate releases (1.2 -> 2.4 GHz) before the real matmuls ----
    pw = p_2.tile([P, P], F32, tag="pg")
    for _ in range(N_WARM):
        nc.tensor.matmul(pw, lhsT=warm_sb, rhs=warm_sb, start=True, stop=True)

    if stage <= 1:
        return dbg(msk_sb[:, 0:4, :])

    # ---- phase 1: masks transposes (+msum partials) ----
    mskT = big.tile([P, GROUPS, P + 1], BF16)  # [32j+k, g, p | msum partial]
    for g in range(GROUPS):
        mview = msk_sb[:, g * CPG:(g + 1) * CPG, :].rearrange("p a b -> p (a b)")
        pt = p_mt.tile([P, P + 1], F32)
        nc.tensor.matmul(pt, lhsT=mview, rhs=id129, start=True, stop=True)
        nc.vector.tensor_copy(out=mskT[:, g, :], in_=pt)
    if stage <= 2:
        return dbg(mskT[:, 0, :])

    # ---- msum -> recip (masks-only; runs while emb still streams) ----
    msum_parts = mskT[:, :, P:P + 1].rearrange("p g o -> p (g o)")  # [P, 8]
    msum_big = small.tile([P, 1], BF16)   # counts <= ~200, exact in bf16
    with nc.allow_low_precision(reason="per-group counts are small ints"):
        nc.vector.reduce_sum(out=msum_big, in_=msum_parts, axis=AX.X)
    ms_psum = p_sm.tile([K, 1], F32, tag="sm")
    nc.tensor.matmul(ms_psum, lhsT=stki, rhs=msum_big, start=True, stop=True)
    recip = small.tile([K, 1], F32)
    nc.vector.reciprocal(recip, ms_psum)
    if stage <= 4:
        return dbg(recip)

    # ---- Cu accumulation: 16 chunk-pair matmuls into one [64,128] psum.
    # lhsT = [msk_2i | msk_2i+1] [128,64], rhs = [emb_2i | emb_2i+1]
    # [128,128]; the TL [0:32,0:64] and BR [32:64,64:128] blocks hold the
    # even/odd-chunk Cu partials (cross blocks are junk). ----
    cu_psum = p_cu.tile([2 * K, P], F32)
    for i in range(NPAIR):
        nc.tensor.matmul(
            cu_psum,
            lhsT=msk_sb[:, 2 * i:2 * i + 2, :].rearrange("p a b -> p (a b)"),
            rhs=emb_sb[:, i * 2 * E:(i + 1) * 2 * E],
            start=(i == 0),
            stop=(i == NPAIR - 1),
        )
    cu_bf = small.tile([2 * K, P], BF16)
    nc.vector.tensor_copy(out=cu_bf, in_=cu_psum)
    # fold TL+BR via two row-selection matmuls ([I32;0] then [0;I32])
    c_psum = p_sm.tile([K, E], F32, tag="sm")
    nc.tensor.matmul(c_psum, lhsT=id129[0:2 * K, 0:K], rhs=cu_bf[:, 0:E],
                     start=True, stop=False)
    nc.tensor.matmul(c_psum, lhsT=id129[0:2 * K, K:2 * K], rhs=cu_bf[:, E:P],
                     start=False, stop=True)
    if stage == 44:
        return dbg(c_psum)

    # ---- C = Cu * recip; cn2 ----
    c_bf = small.tile([K, E], BF16)
    nc.vector.tensor_scalar_mul(c_bf, in0=c_psum, scalar1=recip)
    if stage == 45:
        return dbg(c_bf)
    scr_ke = small.tile([K, E], BF16)
    cn2 = small.tile([K, 1], F32)
    nc.scalar.activation(scr_ke, c_bf, AF.Square, accum_out=cn2)
    if stage <= 5:
        return dbg(c_bf)

    # ---- block-diag C: PE-replicate C 4x vertically, then 4 lane-aligned
    # copies into the diagonal blocks (partition ranges match, no DMA).
    rep_psum = p_sm.tile([P, E], F32, tag="sm")
    nc.tensor.matmul(rep_psum, lhsT=stkit, rhs=c_bf, start=True, stop=True)
    for j in range(CPG):
        dst = c4bd[j * K:(j + 1) * K, j * E:(j + 1) * E]
        src = rep_psum[j * K:(j + 1) * K, :]
        if j % 2 == 0:
            nc.vector.tensor_copy(out=dst, in_=src)
        else:
            nc.scalar.copy(out=dst, in_=src)
    if stage <= 6:
        return dbg(c4bd)

    # ---- phase 4: per-group diff on PE, square on ACT/GpSimd, reduce on
    # DVE into dist2 [p, 32] ----
    dist2 = small.tile([P, CHUNKS], F32)
    for g in range(GROUPS):
        pg = p_2.tile([P, CPG * E], F32, tag="pg")
        nc.tensor.matmul(
            pg, lhsT=mskT[:, g, 0:P], rhs=c4bd, start=True, stop=False
        )
        nc.tensor.matmul(
            pg, lhsT=negi, rhs=emb_sb[:, g * CPG * E:(g + 1) * CPG * E],
            start=False, stop=True,
        )
        sq_g = work.tile([P, CPG * E], BF16, tag="sq")
        nc.scalar.activation(sq_g, pg, AF.Square)
        nc.vector.reduce_sum(
            out=dist2[:, g * CPG:(g + 1) * CPG],
            in_=sq_g.rearrange("p (a b) -> p a b", b=E),
            axis=AX.X,
        )
    if stage <= 8:
        return dbg(dist2)

    # ---- tiny pairwise-centroid tail (L_d, L_r); concurrent with phase 4 ----
    ct_psum = p_sm.tile([E, K], F32, tag="sm")
    nc.tensor.matmul(ct_psum, lhsT=c_bf, rhs=id129[0:K, 0:K],
                     start=True, stop=True)
    ct_sb = small.tile([E, K], BF16)
    nc.vector.tensor_copy(out=ct_sb, in_=ct_psum)
    g_psum = p_sm.tile([K, K], F32, tag="sm")
    nc.tensor.matmul(g_psum, lhsT=ct_sb, rhs=ct_sb, start=True, stop=True)
    w_sb = small.tile([K, K], BF16)
    nc.vector.tensor_scalar(
        out=w_sb, in0=g_psum, scalar1=-2.0, scalar2=cn2,
        op0=OP.mult, op1=OP.add,
    )
    wt_psum = p_sm.tile([K, K], F32, tag="sm")
    nc.tensor.matmul(wt_psum, lhsT=w_sb, rhs=id129[0:K, 0:K],
                     start=True, stop=True)
    d2_sb = small.tile([K, K], F32)
    nc.vector.tensor_scalar(
        out=d2_sb, in0=wt_psum, scalar1=cn2, scalar2=0.0,
        op0=OP.add, op1=OP.max,
    )
    d_sb = small.tile([K, K], F32)
    nc.scalar.sqrt(d_sb, d2_sb)
    h0_sb = small.tile([K, K], F32)
    nc.scalar.activation(h0_sb, d_sb, AF.Relu, bias=bias_m, scale=-1.0)
    h_sb = small.tile([K, K], F32)
    nc.vector.tensor_mul(h_sb, h0_sb, eyec)
    scr_kk = small.tile([K, K], F32)
    ld_raw = small.tile([K, 1], F32)
    nc.scalar.activation(scr_kk, h_sb, AF.Square, accum_out=ld_raw)
    cr_row = small.tile([K, 1], F32)
    nc.scalar.activation(cr_row, cn2, AF.Sqrt, scale=(GAMMA / K) ** 2)
    nc.vector.tensor_scalar(
        out=init_acc[0:K, :], in0=ld_raw, scalar1=BETA / float(K * (K - 1)),
        scalar2=cr_row, op0=OP.mult, op1=OP.add,
    )
    if stage <= 7:
        return dbg(init_acc)

    # ---- variance hinge + final reduction ----
    s_sb = small.tile([P, CHUNKS], F32)
    nc.scalar.sqrt(s_sb, dist2)
    hv_sb = small.tile([P, CHUNKS], F32)
    nc.vector.tensor_scalar(
        out=hv_sb, in0=s_sb, scalar1=DELTA_V, scalar2=0.0,
        op0=OP.subtract, op1=OP.max,
    )
    scr_v = small.tile([P, CHUNKS], BF16)
    tall_raw = small.tile([P, 1], F32)
    nc.scalar.activation(scr_v, hv_sb, AF.Square, accum_out=tall_raw)
    tall = small.tile([P, 1], F32)
    nc.vector.tensor_scalar(
        out=tall, in0=tall_raw, scalar1=ALPHA / float(N),
        scalar2=init_acc, op0=OP.mult, op1=OP.add,
    )
    f_psum = p_sm.tile([1, 1], F32, tag="sm")
    nc.tensor.matmul(f_psum, lhsT=tall, rhs=ones1, start=True, stop=True)
    out_sb = small.tile([1, 1], F32)
    nc.scalar.copy(out_sb, f_psum)
    nc.sync.dma_start(out=t["out"][:, :], in_=out_sb)


def build_nc(stage=99):
    nc = bacc.Bacc("TRN2", target_bir_lowering=False, debug=False)
    t = {
        "emb": nc.dram_tensor("emb", [N, E], BF16, kind="ExternalInput"),
        "msk": nc.dram_tensor("msk", [N, K], BF16, kind="ExternalInput"),
        "cpack": nc.dram_tensor("cpack", [P, CP_W], BF16, kind="ExternalInput"),
        "out": nc.dram_tensor("out", [1, 1], F32, kind="ExternalOutput"),
    }
    if stage < 99:
        t["dbg"] = nc.dram_tensor("dbg", [P, 2048], F32, kind="ExternalOutput")

    with tile.TileContext(nc) as tc, ExitStack() as ctx:
        _body(nc, tc, ctx, t, stage)

    nc.compile()
    return nc


def host_consts():
    cpack = np.zeros((P, CP_W), dtype=ml_dtypes.bfloat16)
    cpack[:, 0:P] = np.eye(P)
    cpack[:, P] = 1.0
    cpack[:, CP_STKI:CP_STKI + K] = np.tile(np.eye(K), (CPG, 1))
    cpack[0:K, CP_EYEC:CP_EYEC + K] = 1.0 - np.eye(K)
    cpack[0:K, CP_STKIT:CP_STKIT + P] = np.tile(np.eye(K), (1, CPG))
    cpack[:, CP_NEGI:CP_NEGI + P] = -np.eye(P)
    return cpack


def make_in_maps(embedded, masks):
    emb = np.asarray(embedded).astype(ml_dtypes.bfloat16)
    msk = np.asarray(masks).astype(ml_dtypes.bfloat16)
    cpack = host_consts()
    return [
        {"emb": np.ascontiguousarray(emb[i]),
         "msk": np.ascontiguousarray(msk[i]),
         "cpack": cpack}
        for i in range(B)
    ]


_NC = None


def _get_nc():
    global _NC
    if _NC is None:
        _NC = build_nc()
    return _NC


def _install_ntff_shim():
    """Register the axon NTFF profile hook if the image's antenv lacks it."""
    import sys as _sys
    import types as _types

    try:
        from antenv.axon_hooks import get_axon_ntff_profile_hook  # noqa: F401
        return
    except ImportError:
        pass
    try:
        from trn_agent_boot.trn_boot import _ntff_profile_via_ctypes

        hook = _ntff_profile_via_ctypes("/opt/axon/libaxon_pjrt.so")
        mod = _types.ModuleType("antenv.axon_hooks")
        mod.get_axon_ntff_profile_hook = lambda: hook
        mod.set_axon_ntff_profile_hook = lambda h: None
        _sys.modules["antenv.axon_hooks"] = mod
    except Exception:
        pass


def run(embedded, masks, trace=False):
    nc = _get_nc()
    if trace:
        _install_ntff_shim()
    res = bass_utils.run_bass_kernel_spmd(
        nc, make_in_maps(embedded, masks), core_ids=list(range(B)), trace=trace
    )
    vals = np.array([r["out"][0, 0] for r in res.results], dtype=np.float64)
    return np.asarray(vals.mean(), dtype=np.float32), res


def kernel(embedded, masks, size):
    out, _ = run(embedded, masks)
    return out
